# revision 1
# baseline (speedup 1.0000x reference)
"""Multi-head attention block (B=4, S=2048, D=1024, H=16) on 8 TRN2 NeuronCores.

Sharding: core c handles batch b = c//2 and head-group hg = c%2 (8 heads,
a 512-wide slice of the qkv projections). No collectives: each core
computes a [D, S] transposed partial of the output projection for its
head group; the host sums the two head-group partials per batch, adds
the output bias, and transposes back to [S, D].

Per-core dataflow (bf16 compute, f32 PSUM accumulation):
  - host pre-casts all big inputs to bf16 (so the device does no casting)
  - xbar transpose-DMA loads X^T [din, s] straight from DRAM
  - Q^T/K^T from projections (dout on partitions); biases folded in as
    ones (x) bias rank-1 matmul updates
  - V in natural [s, dout] layout, augmented with a ones column per head
    (softmax denominators ride along the attn@V matmul as a 65th row)
  - scores^T [k, q] per head via row-packed (tile_position) K=64 matmul
    pairs; exp on ACT (PSUM -> SBUF bf16, scale=1/8 folded in); O_aug
    accumulated over k tiles in PSUM; normalization via DVE reciprocal +
    GPSIMD partition-broadcast + DVE multiply
  - out^T = Wo^T O^T -> [D, S] f32 -> DMA out
"""

import numpy as np
import ml_dtypes

import concourse.bass as bass
import concourse.bacc as bacc
import concourse.mybir as mybir
from concourse.tile import TileContext
from concourse.bass import ds

F32 = mybir.dt.float32
BF16 = mybir.dt.bfloat16
EXP = mybir.ActivationFunctionType.Exp

B, S, D, H, HD = 4, 2048, 1024, 16, 64
N_CORES = 8
HPC = H // (N_CORES // B)          # heads per core = 8
DV = HPC * HD                      # 512


def build_attn_core(S=2048, D=1024, HPC=8, HD=64):
    DV = HPC * HD            # head-group width
    NPAIR = HPC // 2         # head pairs; DV = NPAIR * 128
    NDT = D // 128           # din tiles
    NKT = S // 128           # key tiles
    QC = 512                 # q chunk
    NQC = S // QC
    SC = 512                 # s chunk for projections
    NSC = S // SC
    SCALE = HD ** -0.5

    nc = bacc.Bacc("TRN2", target_bir_lowering=False)
    q_ext = nc.dram_tensor("query", [S, D], BF16, kind="ExternalInput")
    k_ext = nc.dram_tensor("key", [S, D], BF16, kind="ExternalInput")
    v_ext = nc.dram_tensor("value", [S, D], BF16, kind="ExternalInput")
    wq_ext = nc.dram_tensor("Wq", [D, DV], BF16, kind="ExternalInput")
    wk_ext = nc.dram_tensor("Wk", [D, DV], BF16, kind="ExternalInput")
    wv_ext = nc.dram_tensor("Wv", [D, DV], BF16, kind="ExternalInput")
    wo_ext = nc.dram_tensor("Wo", [DV, D], BF16, kind="ExternalInput")
    bq_ext = nc.dram_tensor("bq", [DV], BF16, kind="ExternalInput")
    bk_ext = nc.dram_tensor("bk", [DV], BF16, kind="ExternalInput")
    bv_ext = nc.dram_tensor("bv", [DV], BF16, kind="ExternalInput")
    out_ext = nc.dram_tensor("out", [D, S], F32, kind="ExternalOutput")

    with TileContext(nc) as tc:
        with (
            tc.tile_pool(name="const", bufs=1) as cpool,
            tc.tile_pool(name="big", bufs=1) as big,
            tc.tile_pool(name="pt", bufs=4) as ptpool,
            tc.tile_pool(name="rec", bufs=2) as recpool,
            tc.tile_pool(name="stage", bufs=3) as stage,
            tc.tile_pool(name="mmps", bufs=2, space="PSUM") as mmps,
            tc.tile_pool(name="scps", bufs=2, space="PSUM") as scps,
            tc.tile_pool(name="ops", bufs=2, space="PSUM") as opool,
        ):
            # ---------------- input transpose (xbar, from DRAM bf16) --------
            xqT = big.tile([128, NDT, S], BF16, tag="xqT")
            xkT = big.tile([128, NDT, S], BF16, tag="xkT")
            xvT = big.tile([128, NDT, S], BF16, tag="xqT")  # reuse xqT memory
            for dt in range(NDT):
                nc.sync.dma_start_transpose(xqT[:, dt, :], q_ext[:, ds(dt * 128, 128)])
            for dt in range(NDT):
                nc.sync.dma_start_transpose(xkT[:, dt, :], k_ext[:, ds(dt * 128, 128)])
            for dt in range(NDT):
                nc.sync.dma_start_transpose(xvT[:, dt, :], v_ext[:, ds(dt * 128, 128)])

            # ---------------- weights / biases / constants ----------------
            wq_sb = big.tile([128, NDT, DV], BF16, tag="wq")
            wk_sb = big.tile([128, NDT, DV], BF16, tag="wk")
            wv_sb = big.tile([128, NDT, DV], BF16, tag="wv")
            nc.sync.dma_start(wq_sb[:], wq_ext.rearrange("(t p) n -> p t n", p=128))
            nc.sync.dma_start(wk_sb[:], wk_ext.rearrange("(t p) n -> p t n", p=128))
            nc.sync.dma_start(wv_sb[:], wv_ext.rearrange("(t p) n -> p t n", p=128))

            bq_row = cpool.tile([1, DV], BF16, tag="bqr")
            bk_row = cpool.tile([1, DV], BF16, tag="bkr")
            bv_row = cpool.tile([1, DV], BF16, tag="bv")
            nc.sync.dma_start(bq_row[:], bq_ext.rearrange("(a n) -> a n", a=1))
            nc.sync.dma_start(bk_row[:], bk_ext.rearrange("(a n) -> a n", a=1))
            nc.sync.dma_start(bv_row[:], bv_ext.rearrange("(a n) -> a n", a=1))

            ones128 = cpool.tile([1, 128], BF16, tag="ones128")
            nc.vector.memset(ones128[:], 1.0)
            ones512 = cpool.tile([1, SC], BF16, tag="ones512")
            nc.vector.memset(ones512[:], 1.0)

            # ---------------- Q projection (all pairs up front) ------------
            qT = big.tile([128, NPAIR, S], BF16, tag="qT")
            kT = big.tile([128, NPAIR, S], BF16, tag="kT")
            for t in range(NPAIR):
                for sc in range(NSC):
                    ps = mmps.tile([128, SC], F32, tag="mm")
                    for dk in range(NDT):
                        nc.tensor.matmul(
                            ps[:],
                            wq_sb[:, dk, ds(t * 128, 128)],
                            xqT[:, dk, ds(sc * SC, SC)],
                            start=(dk == 0),
                            stop=False,
                        )
                    nc.tensor.matmul(
                        ps[:], bq_row[:, ds(t * 128, 128)], ones512[:],
                        start=False, stop=True,
                    )
                    nc.vector.tensor_copy(qT[:, t, ds(sc * SC, SC)], ps[:])

            def k_proj(t):
                for sc in range(NSC):
                    ps = mmps.tile([128, SC], F32, tag="mm")
                    for dk in range(NDT):
                        nc.tensor.matmul(
                            ps[:],
                            wk_sb[:, dk, ds(t * 128, 128)],
                            xkT[:, dk, ds(sc * SC, SC)],
                            start=(dk == 0),
                            stop=False,
                        )
                    nc.tensor.matmul(
                        ps[:], bk_row[:, ds(t * 128, 128)], ones512[:],
                        start=False, stop=True,
                    )
                    nc.vector.tensor_copy(kT[:, t, ds(sc * SC, SC)], ps[:])

            k_proj(0)

            # ---------------- V projection (natural layout, augmented) -----
            v_aug = big.tile([128, NKT, HPC * 65], BF16, tag="vaug")
            for st in range(NKT):
                ps = mmps.tile([128, DV], F32, tag="mm")
                for dk in range(NDT):
                    nc.tensor.matmul(
                        ps[:],
                        xvT[:, dk, ds(st * 128, 128)],
                        wv_sb[:, dk, :],
                        start=(dk == 0),
                        stop=False,
                    )
                # bias as rank-1 update: ones[s] x bv[dout]
                nc.tensor.matmul(ps[:], ones128[:], bv_row[:], start=False, stop=True)
                dst = v_aug[:, st, :].rearrange("p (h c) -> p h c", c=65)
                nc.vector.tensor_copy(
                    dst[:, :, 0:64], ps[:].rearrange("p (h c) -> p h c", c=64)
                )
                nc.vector.memset(dst[:, :, 64:65], 1.0)

            # load Wo later to reduce early SBUF pressure
            wo_sb = big.tile([128, NPAIR, D], BF16, tag="wo")
            nc.sync.dma_start(wo_sb[:], wo_ext.rearrange("(t p) n -> p t n", p=128))

            # ---------------- attention per head pair ----------------
            oT = big.tile([128, NPAIR, S], BF16, tag="oT")
            for t in range(NPAIR):
                for qc in range(NQC):
                    oA = opool.tile([65, QC], F32, tag="o")
                    oB = opool.tile([65, QC], F32, tag="o")
                    for kt in range(NKT):
                        sct = scps.tile([128, 2 * QC], F32, tag="sc")
                        nc.tensor.matmul(
                            sct[:, 0:QC],
                            kT[0:64, t, ds(kt * 128, 128)],
                            qT[0:64, t, ds(qc * QC, QC)],
                            start=True,
                            stop=True,
                            tile_position=(0, 0),
                        )
                        nc.tensor.matmul(
                            sct[:, QC : 2 * QC],
                            kT[64:128, t, ds(kt * 128, 128)],
                            qT[64:128, t, ds(qc * QC, QC)],
                            start=True,
                            stop=True,
                            tile_position=(64, 0),
                        )
                        pt = ptpool.tile([128, 2 * QC], BF16, tag="pt")
                        nc.scalar.activation(pt[:], sct[:], EXP, bias=0.0, scale=SCALE)
                        nc.tensor.matmul(
                            oA[:],
                            v_aug[:, kt, ds((2 * t) * 65, 65)],
                            pt[:, 0:QC],
                            start=(kt == 0),
                            stop=(kt == NKT - 1),
                        )
                        nc.tensor.matmul(
                            oB[:],
                            v_aug[:, kt, ds((2 * t + 1) * 65, 65)],
                            pt[:, QC : 2 * QC],
                            start=(kt == 0),
                            stop=(kt == NKT - 1),
                        )
                    for o_ps, hh in ((oA, 0), (oB, 1)):
                        rec_full = recpool.tile([64, QC], F32, tag="rec")
                        rec = rec_full[0:1, :]
                        nc.vector.reciprocal(rec[:], o_ps[64:65, :])
                        bc = recpool.tile([64, QC], F32, tag="rec")
                        nc.gpsimd.partition_broadcast(bc[:], rec[:])
                        nc.vector.tensor_mul(
                            oT[ds(hh * 64, 64), t, ds(qc * QC, QC)],
                            o_ps[0:64, :],
                            bc[:],
                        )
                if t + 1 < NPAIR:
                    k_proj(t + 1)

            # ---------------- output projection (transposed) ----------------
            for dt2 in range(NDT):
                for sc in range(NSC):
                    po = mmps.tile([128, SC], F32, tag="mm")
                    for ht in range(NPAIR):
                        nc.tensor.matmul(
                            po[:],
                            wo_sb[:, ht, ds(dt2 * 128, 128)],
                            oT[:, ht, ds(sc * SC, SC)],
                            start=(ht == 0),
                            stop=(ht == NPAIR - 1),
                        )
                    ost = stage.tile([128, SC], F32, tag="ostage")
                    nc.vector.tensor_copy(ost[:], po[:])
                    nc.sync.dma_start(
                        out_ext[ds(dt2 * 128, 128), ds(sc * SC, SC)], ost[:]
                    )

    nc.finalize()
    return nc


_NC_CACHE = {}


def _get_nc():
    if "nc" not in _NC_CACHE:
        _NC_CACHE["nc"] = build_attn_core(S=S, D=D, HPC=HPC, HD=HD)
    return _NC_CACHE["nc"]


def _make_in_maps(query, key, value, Wq, bq, Wk, bk, Wv, bv, Wo):
    bf = ml_dtypes.bfloat16
    in_maps = []
    for c in range(N_CORES):
        b, hg = c // 2, c % 2
        sl = slice(hg * DV, (hg + 1) * DV)
        in_maps.append(dict(
            query=np.ascontiguousarray(query[b]).astype(bf),
            key=np.ascontiguousarray(key[b]).astype(bf),
            value=np.ascontiguousarray(value[b]).astype(bf),
            Wq=np.ascontiguousarray(Wq[:, sl]).astype(bf),
            Wk=np.ascontiguousarray(Wk[:, sl]).astype(bf),
            Wv=np.ascontiguousarray(Wv[:, sl]).astype(bf),
            Wo=np.ascontiguousarray(Wo[sl, :]).astype(bf),
            bq=np.ascontiguousarray(bq[sl]).astype(bf),
            bk=np.ascontiguousarray(bk[sl]).astype(bf),
            bv=np.ascontiguousarray(bv[sl]).astype(bf),
        ))
    return in_maps


def _assemble(results, bo):
    out = np.empty((B, S, D), dtype=np.float32)
    for b in range(B):
        part = results[2 * b]["out"] + results[2 * b + 1]["out"]   # [D, S]
        out[b] = part.T + bo
    return out


def run(inputs, trace=False):
    """Run on 8 cores; returns (output, BassKernelResults)."""
    from concourse.bass_utils import run_bass_kernel_spmd

    inputs = {k: np.asarray(v) for k, v in inputs.items()}
    nc = _get_nc()
    in_maps = _make_in_maps(
        inputs["query"], inputs["key"], inputs["value"],
        inputs["Wq"], inputs["bq"], inputs["Wk"], inputs["bk"],
        inputs["Wv"], inputs["bv"], inputs["Wo"],
    )
    res = run_bass_kernel_spmd(
        nc, in_maps, core_ids=list(range(N_CORES)), trace=trace
    )
    out = _assemble(res.results, np.asarray(inputs["bo"], dtype=np.float32))
    return out, res


def kernel(**inputs) -> np.ndarray:
    out, _ = run(inputs, trace=False)
    return out


# revision 3
# speedup vs baseline: 1.0207x; 1.0207x over previous
"""Multi-head attention block (B=4, S=2048, D=1024, H=16) on 8 TRN2 NeuronCores.

Sharding: core c handles batch b = c//2 and head-group hg = c%2 (8 heads,
a 512-wide slice of the qkv projections). No collectives: each core
computes a [D, S] transposed partial of the output projection for its
head group; the host sums the two head-group partials per batch, adds
the output bias, and transposes back to [S, D].

Per-core dataflow (bf16 compute, f32 PSUM accumulation):
  - host pre-casts all big inputs to bf16 (so the device does no casting)
  - xbar transpose-DMA loads X^T [din, s] straight from DRAM
  - Q^T/K^T from projections (dout on partitions); biases folded in as
    ones (x) bias rank-1 matmul updates
  - V in natural [s, dout] layout, augmented with a ones column per head
    (softmax denominators ride along the attn@V matmul as a 65th row)
  - scores^T [k, q] per head via zero-padded K=128 matmuls (uniform
    128x128 tile mode); exp on ACT (PSUM -> SBUF bf16, scale=1/8); O_aug
    accumulated over k tiles in PSUM; normalization via DVE reciprocal +
    GPSIMD partition-broadcast + DVE multiply
  - out^T = Wo^T O^T -> [D, S] f32 -> DMA out
"""

import numpy as np
import ml_dtypes

import concourse.bass as bass
import concourse.bacc as bacc
import concourse.mybir as mybir
from concourse.tile import TileContext
from concourse.bass import ds

F32 = mybir.dt.float32
BF16 = mybir.dt.bfloat16
EXP = mybir.ActivationFunctionType.Exp

B, S, D, H, HD = 4, 2048, 1024, 16, 64
N_CORES = 8
HPC = H // (N_CORES // B)          # heads per core = 8
DV = HPC * HD                      # 512


def build_attn_core(S=2048, D=1024, HPC=8, HD=64):
    DV = HPC * HD            # head-group width
    NPAIR = HPC // 2         # head pairs; DV = NPAIR * 128
    NDT = D // 128           # din tiles
    NKT = S // 128           # key tiles
    QC = 512                 # q chunk
    NQC = S // QC
    SC = 512                 # s chunk for projections
    NSC = S // SC
    SCALE = HD ** -0.5

    nc = bacc.Bacc("TRN2", target_bir_lowering=False)
    q_ext = nc.dram_tensor("query", [S, D], BF16, kind="ExternalInput")
    k_ext = nc.dram_tensor("key", [S, D], BF16, kind="ExternalInput")
    v_ext = nc.dram_tensor("value", [S, D], BF16, kind="ExternalInput")
    wq_ext = nc.dram_tensor("Wq", [D, DV], BF16, kind="ExternalInput")
    wk_ext = nc.dram_tensor("Wk", [D, DV], BF16, kind="ExternalInput")
    wv_ext = nc.dram_tensor("Wv", [D, DV], BF16, kind="ExternalInput")
    wo_ext = nc.dram_tensor("Wo", [DV, D], BF16, kind="ExternalInput")
    bq_ext = nc.dram_tensor("bq", [DV], BF16, kind="ExternalInput")
    bk_ext = nc.dram_tensor("bk", [DV], BF16, kind="ExternalInput")
    bv_ext = nc.dram_tensor("bv", [DV], BF16, kind="ExternalInput")
    out_ext = nc.dram_tensor("out", [D, S], F32, kind="ExternalOutput")

    with TileContext(nc) as tc:
        with (
            tc.tile_pool(name="const", bufs=1) as cpool,
            tc.tile_pool(name="big", bufs=1) as big,
            tc.tile_pool(name="pt", bufs=4) as ptpool,
            tc.tile_pool(name="rec", bufs=2) as recpool,
            tc.tile_pool(name="stage", bufs=3) as stage,
            tc.tile_pool(name="mmps", bufs=2, space="PSUM") as mmps,
            tc.tile_pool(name="scps", bufs=2, space="PSUM") as scps,
            tc.tile_pool(name="ops", bufs=2, space="PSUM") as opool,
        ):
            # ---------------- input transpose (xbar, from DRAM bf16) --------
            xqT = big.tile([128, NDT, S], BF16, tag="xqT")
            xkT = big.tile([128, NDT, S], BF16, tag="xkT")
            xvT = big.tile([128, NDT, S], BF16, tag="xqT")  # reuse xqT memory
            for dt in range(NDT):
                nc.sync.dma_start_transpose(xqT[:, dt, :], q_ext[:, ds(dt * 128, 128)])
            for dt in range(NDT):
                nc.sync.dma_start_transpose(xkT[:, dt, :], k_ext[:, ds(dt * 128, 128)])
            for dt in range(NDT):
                nc.sync.dma_start_transpose(xvT[:, dt, :], v_ext[:, ds(dt * 128, 128)])

            # ---------------- weights / biases / constants ----------------
            wq_sb = big.tile([128, NDT, DV], BF16, tag="wq")
            wk_sb = big.tile([128, NDT, DV], BF16, tag="wk")
            wv_sb = big.tile([128, NDT, DV], BF16, tag="wv")
            nc.sync.dma_start(wq_sb[:], wq_ext.rearrange("(t p) n -> p t n", p=128))
            nc.sync.dma_start(wk_sb[:], wk_ext.rearrange("(t p) n -> p t n", p=128))
            nc.sync.dma_start(wv_sb[:], wv_ext.rearrange("(t p) n -> p t n", p=128))

            # biases / ones, zero-padded to 128 partitions so every matmul
            # runs in the same 128x128 tile mode (no mode-switch drains)
            bq_pad = cpool.tile([128, DV], BF16, tag="bqp")
            bk_pad = cpool.tile([128, DV], BF16, tag="bkp")
            bv_pad = cpool.tile([128, DV], BF16, tag="bvp")
            ones_pad = cpool.tile([128, SC], BF16, tag="onesp")
            nc.vector.memset(bq_pad[:], 0.0)
            nc.vector.memset(bk_pad[:], 0.0)
            nc.vector.memset(bv_pad[:], 0.0)
            nc.vector.memset(ones_pad[:], 0.0)
            nc.vector.memset(ones_pad[0:1, :], 1.0)
            nc.sync.dma_start(bq_pad[0:1, :], bq_ext.rearrange("(a n) -> a n", a=1))
            nc.sync.dma_start(bk_pad[0:1, :], bk_ext.rearrange("(a n) -> a n", a=1))
            nc.sync.dma_start(bv_pad[0:1, :], bv_ext.rearrange("(a n) -> a n", a=1))

            # ---------------- Q/K projections (emitted per pair below) ------
            # kT is stored twice with complementary halves zeroed, so the
            # scores matmuls can use full K=128 operands (uniform 128x128
            # tile mode -> no TensorE mode-switch drains): the zero rows of
            # the stationary operand nullify the other head's contribution.
            qT = big.tile([128, NPAIR, S], BF16, tag="qT")
            kTe = big.tile([128, NPAIR, S], BF16, tag="kTe")  # even heads, rows 64:128 zero
            kTo = big.tile([128, NPAIR, S], BF16, tag="kTo")  # odd heads, rows 0:64 zero
            nc.gpsimd.memset(kTe[64:128, :, :], 0.0)
            nc.gpsimd.memset(kTo[0:64, :, :], 0.0)

            def q_proj(t):
                for sc in range(NSC):
                    ps = mmps.tile([128, SC], F32, tag="mm")
                    for dk in range(NDT):
                        nc.tensor.matmul(
                            ps[:],
                            wq_sb[:, dk, ds(t * 128, 128)],
                            xqT[:, dk, ds(sc * SC, SC)],
                            start=(dk == 0),
                            stop=False,
                        )
                    nc.tensor.matmul(
                        ps[:], bq_pad[:, ds(t * 128, 128)], ones_pad[:],
                        start=False, stop=True,
                    )
                    nc.vector.tensor_copy(qT[:, t, ds(sc * SC, SC)], ps[:])

            def k_proj(t):
                for sc in range(NSC):
                    ps = mmps.tile([128, SC], F32, tag="mm")
                    for dk in range(NDT):
                        nc.tensor.matmul(
                            ps[:],
                            wk_sb[:, dk, ds(t * 128, 128)],
                            xkT[:, dk, ds(sc * SC, SC)],
                            start=(dk == 0),
                            stop=False,
                        )
                    nc.tensor.matmul(
                        ps[:], bk_pad[:, ds(t * 128, 128)], ones_pad[:],
                        start=False, stop=True,
                    )
                    nc.vector.tensor_copy(kTe[0:64, t, ds(sc * SC, SC)], ps[0:64, :])
                    nc.vector.tensor_copy(kTo[64:128, t, ds(sc * SC, SC)], ps[64:128, :])

            # All Q projections up front: they overlap the DMA ramp, and
            # xvT reuses xqT's memory so Q-proj must finish before V-proj.
            for t in range(NPAIR):
                q_proj(t)
            k_proj(0)

            # ---------------- V projection (natural layout, augmented) -----
            v_aug = big.tile([128, NKT, HPC * 65], BF16, tag="vaug")
            for st in range(NKT):
                ps = mmps.tile([128, DV], F32, tag="mm")
                for dk in range(NDT):
                    nc.tensor.matmul(
                        ps[:],
                        xvT[:, dk, ds(st * 128, 128)],
                        wv_sb[:, dk, :],
                        start=(dk == 0),
                        stop=False,
                    )
                # bias as rank-1 update: ones[s] x bv[dout] (K padded to 128)
                nc.tensor.matmul(
                    ps[:], ones_pad[:, 0:128], bv_pad[:], start=False, stop=True
                )
                dst = v_aug[:, st, :].rearrange("p (h c) -> p h c", c=65)
                nc.vector.tensor_copy(
                    dst[:, :, 0:64], ps[:].rearrange("p (h c) -> p h c", c=64)
                )
                nc.vector.memset(dst[:, :, 64:65], 1.0)

            # load Wo later to reduce early SBUF pressure
            wo_sb = big.tile([128, NPAIR, D], BF16, tag="wo")
            nc.sync.dma_start(wo_sb[:], wo_ext.rearrange("(t p) n -> p t n", p=128))

            # ---------------- attention per head pair ----------------
            # Software-pipelined emission: attn@V for k-tile kt is emitted
            # AFTER the scores of kt+1, so the in-order PE queue never stalls
            # waiting on exp(kt) while useful score work is available.
            oT = big.tile([128, NPAIR, S], BF16, tag="oT")
            for t in range(NPAIR):
                for qc in range(NQC):
                    oA = opool.tile([65, QC], F32, tag="o")
                    oB = opool.tile([65, QC], F32, tag="o")
                    pts = {}

                    def scores_exp(kt):
                        sct = scps.tile([128, 2 * QC], F32, tag="sc")
                        nc.tensor.matmul(
                            sct[:, 0:QC],
                            kTe[:, t, ds(kt * 128, 128)],
                            qT[:, t, ds(qc * QC, QC)],
                            start=True, stop=True,
                        )
                        nc.tensor.matmul(
                            sct[:, QC : 2 * QC],
                            kTo[:, t, ds(kt * 128, 128)],
                            qT[:, t, ds(qc * QC, QC)],
                            start=True, stop=True,
                        )
                        pt = ptpool.tile([128, 2 * QC], BF16, tag="pt")
                        nc.scalar.activation(pt[:], sct[:], EXP, bias=0.0, scale=SCALE)
                        pts[kt] = pt

                    def attn_v(kt):
                        pt = pts.pop(kt)
                        nc.tensor.matmul(
                            oA[:],
                            v_aug[:, kt, ds((2 * t) * 65, 65)],
                            pt[:, 0:QC],
                            start=(kt == 0),
                            stop=(kt == NKT - 1),
                        )
                        nc.tensor.matmul(
                            oB[:],
                            v_aug[:, kt, ds((2 * t + 1) * 65, 65)],
                            pt[:, QC : 2 * QC],
                            start=(kt == 0),
                            stop=(kt == NKT - 1),
                        )

                    scores_exp(0)
                    for kt in range(1, NKT):
                        scores_exp(kt)
                        attn_v(kt - 1)
                    attn_v(NKT - 1)
                    for o_ps, hh in ((oA, 0), (oB, 1)):
                        rec_full = recpool.tile([64, QC], F32, tag="rec")
                        rec = rec_full[0:1, :]
                        nc.vector.reciprocal(rec[:], o_ps[64:65, :])
                        bc = recpool.tile([64, QC], F32, tag="rec")
                        nc.gpsimd.partition_broadcast(bc[:], rec[:])
                        nc.vector.tensor_mul(
                            oT[ds(hh * 64, 64), t, ds(qc * QC, QC)],
                            o_ps[0:64, :],
                            bc[:],
                        )
                if t + 1 < NPAIR:
                    k_proj(t + 1)

            # ---------------- output projection (transposed) ----------------
            for dt2 in range(NDT):
                for sc in range(NSC):
                    po = mmps.tile([128, SC], F32, tag="mm")
                    for ht in range(NPAIR):
                        nc.tensor.matmul(
                            po[:],
                            wo_sb[:, ht, ds(dt2 * 128, 128)],
                            oT[:, ht, ds(sc * SC, SC)],
                            start=(ht == 0),
                            stop=(ht == NPAIR - 1),
                        )
                    ost = stage.tile([128, SC], F32, tag="ostage")
                    nc.vector.tensor_copy(ost[:], po[:])
                    nc.sync.dma_start(
                        out_ext[ds(dt2 * 128, 128), ds(sc * SC, SC)], ost[:]
                    )

    nc.finalize()
    return nc


_NC_CACHE = {}


def _get_nc():
    if "nc" not in _NC_CACHE:
        _NC_CACHE["nc"] = build_attn_core(S=S, D=D, HPC=HPC, HD=HD)
    return _NC_CACHE["nc"]


def _make_in_maps(query, key, value, Wq, bq, Wk, bk, Wv, bv, Wo):
    bf = ml_dtypes.bfloat16
    in_maps = []
    for c in range(N_CORES):
        b, hg = c // 2, c % 2
        sl = slice(hg * DV, (hg + 1) * DV)
        in_maps.append(dict(
            query=np.ascontiguousarray(query[b]).astype(bf),
            key=np.ascontiguousarray(key[b]).astype(bf),
            value=np.ascontiguousarray(value[b]).astype(bf),
            Wq=np.ascontiguousarray(Wq[:, sl]).astype(bf),
            Wk=np.ascontiguousarray(Wk[:, sl]).astype(bf),
            Wv=np.ascontiguousarray(Wv[:, sl]).astype(bf),
            Wo=np.ascontiguousarray(Wo[sl, :]).astype(bf),
            bq=np.ascontiguousarray(bq[sl]).astype(bf),
            bk=np.ascontiguousarray(bk[sl]).astype(bf),
            bv=np.ascontiguousarray(bv[sl]).astype(bf),
        ))
    return in_maps


def _assemble(results, bo):
    out = np.empty((B, S, D), dtype=np.float32)
    for b in range(B):
        part = results[2 * b]["out"] + results[2 * b + 1]["out"]   # [D, S]
        out[b] = part.T + bo
    return out


def run(inputs, trace=False):
    """Run on 8 cores; returns (output, BassKernelResults)."""
    from concourse.bass_utils import run_bass_kernel_spmd

    inputs = {k: np.asarray(v) for k, v in inputs.items()}
    nc = _get_nc()
    in_maps = _make_in_maps(
        inputs["query"], inputs["key"], inputs["value"],
        inputs["Wq"], inputs["bq"], inputs["Wk"], inputs["bk"],
        inputs["Wv"], inputs["bv"], inputs["Wo"],
    )
    res = run_bass_kernel_spmd(
        nc, in_maps, core_ids=list(range(N_CORES)), trace=trace
    )
    out = _assemble(res.results, np.asarray(inputs["bo"], dtype=np.float32))
    return out, res


def kernel(**inputs) -> np.ndarray:
    out, _ = run(inputs, trace=False)
    return out


# revision 4
# speedup vs baseline: 1.1994x; 1.1751x over previous
"""Multi-head attention block (B=4, S=2048, D=1024, H=16) on 8 TRN2 NeuronCores.

Sharding: core c handles batch b = c//2 and head-group hg = c%2 (8 heads,
a 512-wide slice of the qkv projections). No collectives: each core
computes a [D, S] transposed partial of the output projection for its
head group; the host sums the two head-group partials per batch, adds
the output bias, and transposes back to [S, D].

Per-core dataflow (bf16 compute, f32 PSUM accumulation):
  - host pre-casts all big inputs to bf16 (so the device does no casting)
  - xbar transpose-DMA loads X^T [din, s] straight from DRAM
  - Q^T/K^T from projections (dout on partitions); biases folded in as
    ones (x) bias rank-1 matmul updates
  - V in natural [s, dout] layout, augmented with a ones column per head
    (softmax denominators ride along the attn@V matmul as a 65th row)
  - scores^T [k, q] per head via zero-padded K=128 matmuls (uniform
    128x128 tile mode); exp on ACT (PSUM -> SBUF bf16, scale=1/8); O_aug
    accumulated over k tiles in PSUM; normalization via DVE reciprocal +
    GPSIMD partition-broadcast + DVE multiply
  - out^T = Wo^T O^T -> [D, S] f32 -> DMA out
"""

import numpy as np
import ml_dtypes

import concourse.bass as bass
import concourse.bacc as bacc
import concourse.mybir as mybir
from concourse.tile import TileContext
from concourse.bass import ds

F32 = mybir.dt.float32
BF16 = mybir.dt.bfloat16
EXP = mybir.ActivationFunctionType.Exp

B, S, D, H, HD = 4, 2048, 1024, 16, 64
N_CORES = 8
HPC = H // (N_CORES // B)          # heads per core = 8
DV = HPC * HD                      # 512


def build_attn_core(S=2048, D=1024, HPC=8, HD=64):
    DV = HPC * HD            # head-group width
    NPAIR = HPC // 2         # head pairs; DV = NPAIR * 128
    NDT = D // 128           # din tiles
    NKT = S // 128           # key tiles
    QC = 512                 # q chunk
    NQC = S // QC
    SC = 512                 # s chunk for projections
    NSC = S // SC
    SCALE = HD ** -0.5

    nc = bacc.Bacc("TRN2", target_bir_lowering=False)
    q_ext = nc.dram_tensor("query", [S, D], BF16, kind="ExternalInput")
    k_ext = nc.dram_tensor("key", [S, D], BF16, kind="ExternalInput")
    v_ext = nc.dram_tensor("value", [S, D], BF16, kind="ExternalInput")
    wq_ext = nc.dram_tensor("Wq", [D, DV], BF16, kind="ExternalInput")
    wk_ext = nc.dram_tensor("Wk", [D, DV], BF16, kind="ExternalInput")
    wv_ext = nc.dram_tensor("Wv", [D, DV], BF16, kind="ExternalInput")
    wo_ext = nc.dram_tensor("Wo", [DV, D], BF16, kind="ExternalInput")
    bq_ext = nc.dram_tensor("bq", [DV], BF16, kind="ExternalInput")
    bk_ext = nc.dram_tensor("bk", [DV], BF16, kind="ExternalInput")
    bv_ext = nc.dram_tensor("bv", [DV], BF16, kind="ExternalInput")
    out_ext = nc.dram_tensor("out", [D, S], F32, kind="ExternalOutput")

    with TileContext(nc) as tc:
        with (
            tc.tile_pool(name="const", bufs=1) as cpool,
            tc.tile_pool(name="big", bufs=1) as big,
            tc.tile_pool(name="pt", bufs=4) as ptpool,
            tc.tile_pool(name="rec", bufs=2) as recpool,
            tc.tile_pool(name="oun", bufs=4) as ounpool,
            tc.tile_pool(name="stage", bufs=3) as stage,
            tc.tile_pool(name="mmps", bufs=2, space="PSUM") as mmps,
            tc.tile_pool(name="scps", bufs=2, space="PSUM") as scps,
            tc.tile_pool(name="ops", bufs=2, space="PSUM") as opool,
        ):
            # ---------------- inputs: weights + xbar transposes ------------
            # Emission order = rough DMA priority: everything Q-proj needs
            # first, then K, then V, so the PE ramp starts ~15us in.
            xqT = big.tile([128, NDT, S], BF16, tag="xqT")
            xkT = big.tile([128, NDT, S], BF16, tag="xkT")
            xvT = big.tile([128, NDT, S], BF16, tag="xqT")  # reuse xqT memory
            wq_sb = big.tile([128, NDT, DV], BF16, tag="wq")
            wk_sb = big.tile([128, NDT, DV], BF16, tag="wk")
            wv_sb = big.tile([128, NDT, DV], BF16, tag="wv")
            nc.sync.dma_start(wq_sb[:], wq_ext.rearrange("(t p) n -> p t n", p=128))
            for dt in range(NDT):
                nc.sync.dma_start_transpose(xqT[:, dt, :], q_ext[:, ds(dt * 128, 128)])
            nc.sync.dma_start(wk_sb[:], wk_ext.rearrange("(t p) n -> p t n", p=128))
            for dt in range(NDT):
                nc.sync.dma_start_transpose(xkT[:, dt, :], k_ext[:, ds(dt * 128, 128)])
            nc.sync.dma_start(wv_sb[:], wv_ext.rearrange("(t p) n -> p t n", p=128))
            for dt in range(NDT):
                nc.sync.dma_start_transpose(xvT[:, dt, :], v_ext[:, ds(dt * 128, 128)])

            # biases / ones, zero-padded to 128 partitions so every matmul
            # runs in the same 128x128 tile mode (no mode-switch drains)
            bq_pad = cpool.tile([128, DV], BF16, tag="bqp")
            bk_pad = cpool.tile([128, DV], BF16, tag="bkp")
            bv_pad = cpool.tile([128, DV], BF16, tag="bvp")
            ones_pad = cpool.tile([128, SC], BF16, tag="onesp")
            nc.vector.memset(bq_pad[:], 0.0)
            nc.vector.memset(bk_pad[:], 0.0)
            nc.vector.memset(bv_pad[:], 0.0)
            nc.vector.memset(ones_pad[:], 0.0)
            nc.vector.memset(ones_pad[0:1, :], 1.0)
            nc.sync.dma_start(bq_pad[0:1, :], bq_ext.rearrange("(a n) -> a n", a=1))
            nc.sync.dma_start(bk_pad[0:1, :], bk_ext.rearrange("(a n) -> a n", a=1))
            nc.sync.dma_start(bv_pad[0:1, :], bv_ext.rearrange("(a n) -> a n", a=1))

            # ---------------- Q/K projections (emitted per pair below) ------
            # kT is stored twice with complementary halves zeroed, so the
            # scores matmuls can use full K=128 operands (uniform 128x128
            # tile mode -> no TensorE mode-switch drains): the zero rows of
            # the stationary operand nullify the other head's contribution.
            qT = big.tile([128, NPAIR, S], BF16, tag="qT")
            kTe = big.tile([128, NPAIR, S], BF16, tag="kTe")  # even heads, rows 64:128 zero
            kTo = big.tile([128, NPAIR, S], BF16, tag="kTo")  # odd heads, rows 0:64 zero
            nc.gpsimd.memset(kTe[64:128, :, :], 0.0)
            nc.gpsimd.memset(kTo[0:64, :, :], 0.0)

            def q_proj(t):
                for sc in range(NSC):
                    ps = mmps.tile([128, SC], F32, tag="mm")
                    for dk in range(NDT):
                        nc.tensor.matmul(
                            ps[:],
                            wq_sb[:, dk, ds(t * 128, 128)],
                            xqT[:, dk, ds(sc * SC, SC)],
                            start=(dk == 0),
                            stop=False,
                        )
                    nc.tensor.matmul(
                        ps[:], bq_pad[:, ds(t * 128, 128)], ones_pad[:],
                        start=False, stop=True,
                    )
                    nc.vector.tensor_copy(qT[:, t, ds(sc * SC, SC)], ps[:])

            def k_proj(t):
                for sc in range(NSC):
                    ps = mmps.tile([128, SC], F32, tag="mm")
                    for dk in range(NDT):
                        nc.tensor.matmul(
                            ps[:],
                            wk_sb[:, dk, ds(t * 128, 128)],
                            xkT[:, dk, ds(sc * SC, SC)],
                            start=(dk == 0),
                            stop=False,
                        )
                    nc.tensor.matmul(
                        ps[:], bk_pad[:, ds(t * 128, 128)], ones_pad[:],
                        start=False, stop=True,
                    )
                    nc.vector.tensor_copy(kTe[0:64, t, ds(sc * SC, SC)], ps[0:64, :])
                    nc.vector.tensor_copy(kTo[64:128, t, ds(sc * SC, SC)], ps[64:128, :])

            # All Q projections up front: they overlap the DMA ramp, and
            # xvT reuses xqT's memory so Q-proj must finish before V-proj.
            for t in range(NPAIR):
                q_proj(t)
            k_proj(0)

            # ---------------- V projection (natural layout, augmented) -----
            v_aug = big.tile([128, NKT, HPC * 65], BF16, tag="vaug")
            for st in range(NKT):
                ps = mmps.tile([128, DV], F32, tag="mm")
                for dk in range(NDT):
                    nc.tensor.matmul(
                        ps[:],
                        xvT[:, dk, ds(st * 128, 128)],
                        wv_sb[:, dk, :],
                        start=(dk == 0),
                        stop=False,
                    )
                # bias as rank-1 update: ones[s] x bv[dout] (K padded to 128)
                nc.tensor.matmul(
                    ps[:], ones_pad[:, 0:128], bv_pad[:], start=False, stop=True
                )
                dst = v_aug[:, st, :].rearrange("p (h c) -> p h c", c=65)
                nc.vector.tensor_copy(
                    dst[:, :, 0:64], ps[:].rearrange("p (h c) -> p h c", c=64)
                )
                nc.vector.memset(dst[:, :, 64:65], 1.0)

            # load Wo later to reduce early SBUF pressure
            wo_sb = big.tile([128, NPAIR, D], BF16, tag="wo")
            nc.sync.dma_start(wo_sb[:], wo_ext.rearrange("(t p) n -> p t n", p=128))

            def outproj_sc(sc):
                for dt2 in range(NDT):
                    po = mmps.tile([128, SC], F32, tag="mm")
                    for ht in range(NPAIR):
                        nc.tensor.matmul(
                            po[:],
                            wo_sb[:, ht, ds(dt2 * 128, 128)],
                            oT[:, ht, ds(sc * SC, SC)],
                            start=(ht == 0),
                            stop=(ht == NPAIR - 1),
                        )
                    ost = stage.tile([128, SC], F32, tag="ostage")
                    nc.vector.tensor_copy(ost[:], po[:])
                    nc.sync.dma_start(
                        out_ext[ds(dt2 * 128, 128), ds(sc * SC, SC)], ost[:]
                    )

            # ---------------- attention per head pair ----------------
            # Software-pipelined emission: attn@V for k-tile kt is emitted
            # AFTER the scores of kt+1, so the in-order PE queue never stalls
            # waiting on exp(kt) while useful score work is available.
            oT = big.tile([128, NPAIR, S], BF16, tag="oT")
            for t in range(NPAIR):
                for qc in range(NQC):
                    oA = opool.tile([65, QC], F32, tag="o")
                    oB = opool.tile([65, QC], F32, tag="o")
                    pts = {}

                    def scores_exp(kt):
                        sct = scps.tile([128, 2 * QC], F32, tag="sc")
                        nc.tensor.matmul(
                            sct[:, 0:QC],
                            kTe[:, t, ds(kt * 128, 128)],
                            qT[:, t, ds(qc * QC, QC)],
                            start=True, stop=True,
                        )
                        nc.tensor.matmul(
                            sct[:, QC : 2 * QC],
                            kTo[:, t, ds(kt * 128, 128)],
                            qT[:, t, ds(qc * QC, QC)],
                            start=True, stop=True,
                        )
                        pt = ptpool.tile([128, 2 * QC], BF16, tag="pt")
                        nc.scalar.activation(pt[:], sct[:], EXP, bias=0.0, scale=SCALE)
                        pts[kt] = pt

                    def attn_v(kt):
                        pt = pts.pop(kt)
                        nc.tensor.matmul(
                            oA[:],
                            v_aug[:, kt, ds((2 * t) * 65, 65)],
                            pt[:, 0:QC],
                            start=(kt == 0),
                            stop=(kt == NKT - 1),
                        )
                        nc.tensor.matmul(
                            oB[:],
                            v_aug[:, kt, ds((2 * t + 1) * 65, 65)],
                            pt[:, QC : 2 * QC],
                            start=(kt == 0),
                            stop=(kt == NKT - 1),
                        )

                    scores_exp(0)
                    for kt in range(1, NKT):
                        scores_exp(kt)
                        attn_v(kt - 1)
                    attn_v(NKT - 1)
                    for o_ps, hh in ((oA, 0), (oB, 1)):
                        # Copy PSUM->SBUF immediately (one dep, ~600ns) so the
                        # accumulator bank frees for the next q chunk; the
                        # recip/broadcast/multiply chain then runs off the
                        # critical path on SBUF data.
                        o_sb = ounpool.tile([65, QC], F32, tag="oun")
                        nc.vector.tensor_copy(o_sb[:], o_ps[:])
                        rec_full = recpool.tile([64, QC], F32, tag="rec")
                        rec = rec_full[0:1, :]
                        nc.vector.reciprocal(rec[:], o_sb[64:65, :])
                        bc = recpool.tile([64, QC], F32, tag="rec")
                        nc.gpsimd.partition_broadcast(bc[:], rec[:])
                        nc.vector.tensor_mul(
                            oT[ds(hh * 64, 64), t, ds(qc * QC, QC)],
                            o_sb[0:64, :],
                            bc[:],
                        )
                    if t == NPAIR - 1:
                        # output projection for this s chunk overlaps the
                        # remaining ACT-bound attention work
                        outproj_sc(qc)
                if t + 1 < NPAIR:
                    k_proj(t + 1)


    nc.finalize()
    return nc


_NC_CACHE = {}


def _get_nc():
    if "nc" not in _NC_CACHE:
        _NC_CACHE["nc"] = build_attn_core(S=S, D=D, HPC=HPC, HD=HD)
    return _NC_CACHE["nc"]


def _make_in_maps(query, key, value, Wq, bq, Wk, bk, Wv, bv, Wo):
    bf = ml_dtypes.bfloat16
    in_maps = []
    for c in range(N_CORES):
        b, hg = c // 2, c % 2
        sl = slice(hg * DV, (hg + 1) * DV)
        in_maps.append(dict(
            query=np.ascontiguousarray(query[b]).astype(bf),
            key=np.ascontiguousarray(key[b]).astype(bf),
            value=np.ascontiguousarray(value[b]).astype(bf),
            Wq=np.ascontiguousarray(Wq[:, sl]).astype(bf),
            Wk=np.ascontiguousarray(Wk[:, sl]).astype(bf),
            Wv=np.ascontiguousarray(Wv[:, sl]).astype(bf),
            Wo=np.ascontiguousarray(Wo[sl, :]).astype(bf),
            bq=np.ascontiguousarray(bq[sl]).astype(bf),
            bk=np.ascontiguousarray(bk[sl]).astype(bf),
            bv=np.ascontiguousarray(bv[sl]).astype(bf),
        ))
    return in_maps


def _assemble(results, bo):
    out = np.empty((B, S, D), dtype=np.float32)
    for b in range(B):
        part = results[2 * b]["out"] + results[2 * b + 1]["out"]   # [D, S]
        out[b] = part.T + bo
    return out


def run(inputs, trace=False):
    """Run on 8 cores; returns (output, BassKernelResults)."""
    from concourse.bass_utils import run_bass_kernel_spmd

    inputs = {k: np.asarray(v) for k, v in inputs.items()}
    nc = _get_nc()
    in_maps = _make_in_maps(
        inputs["query"], inputs["key"], inputs["value"],
        inputs["Wq"], inputs["bq"], inputs["Wk"], inputs["bk"],
        inputs["Wv"], inputs["bv"], inputs["Wo"],
    )
    res = run_bass_kernel_spmd(
        nc, in_maps, core_ids=list(range(N_CORES)), trace=trace
    )
    out = _assemble(res.results, np.asarray(inputs["bo"], dtype=np.float32))
    return out, res


def kernel(**inputs) -> np.ndarray:
    out, _ = run(inputs, trace=False)
    return out


# revision 6
# speedup vs baseline: 1.2930x; 1.0780x over previous
"""Multi-head attention block (B=4, S=2048, D=1024, H=16) on 8 TRN2 NeuronCores.

Sharding: core c handles batch b = c//2 and head-group hg = c%2 (8 heads,
a 512-wide slice of the qkv projections). No collectives: each core
computes a [D, S] transposed partial of the output projection for its
head group; the host sums the two head-group partials per batch, adds
the output bias, and transposes back to [S, D].

Per-core dataflow (bf16 compute, f32 PSUM accumulation):
  - host pre-casts all big inputs to bf16 (so the device does no casting)
  - xbar transpose-DMA loads X^T [din, s] straight from DRAM
  - Q^T/K^T from projections (dout on partitions); biases folded in as
    ones (x) bias rank-1 matmul updates
  - V in natural [s, dout] layout, augmented with a ones column per head
    (softmax denominators ride along the attn@V matmul as a 65th row)
  - scores^T [k, q] per head via zero-padded K=128 matmuls (uniform
    128x128 tile mode); exp on ACT (PSUM -> SBUF bf16, scale=1/8); O_aug
    accumulated over k tiles in PSUM; normalization via DVE reciprocal +
    GPSIMD partition-broadcast + DVE multiply
  - out^T = Wo^T O^T -> [D, S] f32 -> DMA out
"""

import numpy as np
import ml_dtypes

import concourse.bass as bass
import concourse.bacc as bacc
import concourse.mybir as mybir
from concourse.tile import TileContext
from concourse.bass import ds

F32 = mybir.dt.float32
BF16 = mybir.dt.bfloat16
EXP = mybir.ActivationFunctionType.Exp

B, S, D, H, HD = 4, 2048, 1024, 16, 64
N_CORES = 8
HPC = H // (N_CORES // B)          # heads per core = 8
DV = HPC * HD                      # 512


def build_attn_core(S=2048, D=1024, HPC=8, HD=64):
    DV = HPC * HD            # head-group width
    NPAIR = HPC // 2         # head pairs; DV = NPAIR * 128
    NDT = D // 128           # din tiles
    NKT = S // 128           # key tiles
    QC = 512                 # q chunk
    NQC = S // QC
    SC = 512                 # s chunk for projections
    NSC = S // SC
    SCALE = HD ** -0.5

    nc = bacc.Bacc("TRN2", target_bir_lowering=False)
    q_ext = nc.dram_tensor("query", [S, D], BF16, kind="ExternalInput")
    k_ext = nc.dram_tensor("key", [S, D], BF16, kind="ExternalInput")
    v_ext = nc.dram_tensor("value", [S, D], BF16, kind="ExternalInput")
    wq_ext = nc.dram_tensor("Wq", [D, DV], BF16, kind="ExternalInput")
    wk_ext = nc.dram_tensor("Wk", [D, DV], BF16, kind="ExternalInput")
    wv_ext = nc.dram_tensor("Wv", [D, DV], BF16, kind="ExternalInput")
    wo_ext = nc.dram_tensor("Wo", [DV, D], BF16, kind="ExternalInput")
    bq_ext = nc.dram_tensor("bq", [DV], BF16, kind="ExternalInput")
    bk_ext = nc.dram_tensor("bk", [DV], BF16, kind="ExternalInput")
    bv_ext = nc.dram_tensor("bv", [DV], BF16, kind="ExternalInput")
    out_ext = nc.dram_tensor("out", [D, S], F32, kind="ExternalOutput")

    with TileContext(nc) as tc:
        with (
            tc.tile_pool(name="const", bufs=1) as cpool,
            tc.tile_pool(name="big", bufs=1) as big,
            tc.tile_pool(name="pt", bufs=4) as ptpool,
            tc.tile_pool(name="rec", bufs=2) as recpool,
            tc.tile_pool(name="oun", bufs=4) as ounpool,
            tc.tile_pool(name="stage", bufs=3) as stage,
            tc.tile_pool(name="mmps", bufs=2, space="PSUM") as mmps,
            tc.tile_pool(name="scps", bufs=2, space="PSUM") as scps,
            tc.tile_pool(name="ops", bufs=2, space="PSUM") as opool,
        ):
            # -------- biases / ones first (tiny DMAs; the last matmul of
            # every projection group needs them, so they must not queue
            # behind the big transfers). Zero-padded to 128 partitions so
            # every matmul runs in the same 128x128 tile mode.
            bq_pad = cpool.tile([128, DV], BF16, tag="bqp")
            bk_pad = cpool.tile([128, DV], BF16, tag="bkp")
            bv_pad = cpool.tile([128, DV], BF16, tag="bvp")
            ones_pad = cpool.tile([128, SC], BF16, tag="onesp")
            nc.vector.memset(bq_pad[:], 0.0)
            nc.vector.memset(bk_pad[:], 0.0)
            nc.vector.memset(bv_pad[:], 0.0)
            nc.vector.memset(ones_pad[:], 0.0)
            nc.vector.memset(ones_pad[0:1, :], 1.0)
            nc.sync.dma_start(bq_pad[0:1, :], bq_ext.rearrange("(a n) -> a n", a=1))
            nc.sync.dma_start(bk_pad[0:1, :], bk_ext.rearrange("(a n) -> a n", a=1))
            nc.sync.dma_start(bv_pad[0:1, :], bv_ext.rearrange("(a n) -> a n", a=1))

            # -------- big inputs: emission order = DMA priority ----------
            xqT = big.tile([128, NDT, S], BF16, tag="xqT")
            xkT = big.tile([128, NDT, S], BF16, tag="xkT")
            xvT = big.tile([128, NDT, S], BF16, tag="xqT")  # reuse xqT memory
            wq_sb = big.tile([128, NDT, DV], BF16, tag="wq")
            wk_sb = big.tile([128, NDT, DV], BF16, tag="wk")
            wv_sb = big.tile([128, NDT, DV], BF16, tag="wv")
            wo_sb = big.tile([128, NPAIR, D], BF16, tag="wo")
            nc.sync.dma_start(wq_sb[:], wq_ext.rearrange("(t p) n -> p t n", p=128))
            for dt in range(NDT):
                nc.sync.dma_start_transpose(xqT[:, dt, :], q_ext[:, ds(dt * 128, 128)])
            nc.sync.dma_start(wk_sb[:], wk_ext.rearrange("(t p) n -> p t n", p=128))
            for dt in range(NDT):
                nc.sync.dma_start_transpose(xkT[:, dt, :], k_ext[:, ds(dt * 128, 128)])
            nc.sync.dma_start(wv_sb[:], wv_ext.rearrange("(t p) n -> p t n", p=128))
            for dt in range(NDT):
                nc.sync.dma_start_transpose(xvT[:, dt, :], v_ext[:, ds(dt * 128, 128)])
            nc.sync.dma_start(wo_sb[:], wo_ext.rearrange("(t p) n -> p t n", p=128))

            # -------- persistent SBUF tensors ----------------------------
            # kT is stored twice with complementary halves zeroed, so the
            # scores matmuls can use full K=128 operands (uniform 128x128
            # tile mode, no mode-switch drains): the zero rows of the
            # stationary operand nullify the other head's contribution.
            qT = big.tile([128, NPAIR, S], BF16, tag="qT")
            kTe = big.tile([128, NPAIR, S], BF16, tag="kTe")
            kTo = big.tile([128, NPAIR, S], BF16, tag="kTo")
            v_aug = big.tile([128, NKT, HPC * 65], BF16, tag="vaug")
            oT = big.tile([128, NPAIR, S], BF16, tag="oT")
            nc.gpsimd.memset(kTe[64:128, :, :], 0.0)
            nc.gpsimd.memset(kTo[0:64, :, :], 0.0)

            # -------- work-chunk emitters --------------------------------
            def q_proj_sc(t, sc):
                ps = mmps.tile([128, SC], F32, tag="mm")
                for dk in range(NDT):
                    nc.tensor.matmul(
                        ps[:],
                        wq_sb[:, dk, ds(t * 128, 128)],
                        xqT[:, dk, ds(sc * SC, SC)],
                        start=(dk == 0),
                        stop=False,
                    )
                nc.tensor.matmul(
                    ps[:], bq_pad[:, ds(t * 128, 128)], ones_pad[:],
                    start=False, stop=True,
                )
                nc.vector.tensor_copy(qT[:, t, ds(sc * SC, SC)], ps[:])

            def k_proj_sc(t, sc):
                ps = mmps.tile([128, SC], F32, tag="mm")
                for dk in range(NDT):
                    nc.tensor.matmul(
                        ps[:],
                        wk_sb[:, dk, ds(t * 128, 128)],
                        xkT[:, dk, ds(sc * SC, SC)],
                        start=(dk == 0),
                        stop=False,
                    )
                nc.tensor.matmul(
                    ps[:], bk_pad[:, ds(t * 128, 128)], ones_pad[:],
                    start=False, stop=True,
                )
                nc.vector.tensor_copy(kTe[0:64, t, ds(sc * SC, SC)], ps[0:64, :])
                nc.vector.tensor_copy(kTo[64:128, t, ds(sc * SC, SC)], ps[64:128, :])

            def v_proj_st(st):
                ps = mmps.tile([128, DV], F32, tag="mm")
                for dk in range(NDT):
                    nc.tensor.matmul(
                        ps[:],
                        xvT[:, dk, ds(st * 128, 128)],
                        wv_sb[:, dk, :],
                        start=(dk == 0),
                        stop=False,
                    )
                nc.tensor.matmul(
                    ps[:], ones_pad[:, 0:128], bv_pad[:], start=False, stop=True
                )
                dst = v_aug[:, st, :].rearrange("p (h c) -> p h c", c=65)
                nc.vector.tensor_copy(
                    dst[:, :, 0:64], ps[:].rearrange("p (h c) -> p h c", c=64)
                )
                nc.vector.memset(dst[:, :, 64:65], 1.0)

            def outproj_dt(sc, dt2):
                po = mmps.tile([128, SC], F32, tag="mm")
                for ht in range(NPAIR):
                    nc.tensor.matmul(
                        po[:],
                        wo_sb[:, ht, ds(dt2 * 128, 128)],
                        oT[:, ht, ds(sc * SC, SC)],
                        start=(ht == 0),
                        stop=(ht == NPAIR - 1),
                    )
                ost = stage.tile([128, SC], F32, tag="ostage")
                nc.vector.tensor_copy(ost[:], po[:])
                nc.sync.dma_start(
                    out_ext[ds(dt2 * 128, 128), ds(sc * SC, SC)], ost[:]
                )

            # -------- projections needed before attention(0) -------------
            for t in range(NPAIR):
                for sc in range(NSC):
                    q_proj_sc(t, sc)
            for sc in range(NSC):
                k_proj_sc(0, sc)

            # -------- attention ------------------------------------------
            # ACT (exp) is the long pole; every other PE-work chunk is
            # interleaved into the kt loop as "filler" so the in-order PE
            # queue never parks a long burst in front of the next scores.
            SCALEF = SCALE

            def attention_pair(t):
                for qc in range(NQC):
                    # filler thunks interleaved after each scores/exp step
                    fillers = {}
                    if t == 0 and qc == 0:
                        for kt in range(NKT):
                            fillers[kt] = (lambda st=kt: v_proj_st(st),)
                    if t + 1 < NPAIR and qc == NQC - 1:
                        for i in range(NSC):
                            fillers[2 + 4 * i] = (
                                lambda tt=t + 1, sc=i: k_proj_sc(tt, sc),
                            )
                    if t == NPAIR - 1 and qc > 0:
                        for i in range(NDT):
                            fillers[2 * i + 1] = (
                                lambda sc=qc - 1, dt2=i: outproj_dt(sc, dt2),
                            )

                    oA = opool.tile([65, QC], F32, tag="o")
                    oB = opool.tile([65, QC], F32, tag="o")
                    pts = {}

                    def scores_exp(kt):
                        sct = scps.tile([128, 2 * QC], F32, tag="sc")
                        nc.tensor.matmul(
                            sct[:, 0:QC],
                            kTe[:, t, ds(kt * 128, 128)],
                            qT[:, t, ds(qc * QC, QC)],
                            start=True, stop=True,
                        )
                        nc.tensor.matmul(
                            sct[:, QC : 2 * QC],
                            kTo[:, t, ds(kt * 128, 128)],
                            qT[:, t, ds(qc * QC, QC)],
                            start=True, stop=True,
                        )
                        pt = ptpool.tile([128, 2 * QC], BF16, tag="pt")
                        nc.scalar.activation(pt[:], sct[:], EXP, bias=0.0, scale=SCALEF)
                        pts[kt] = pt

                    def attn_v(kt):
                        pt = pts.pop(kt)
                        nc.tensor.matmul(
                            oA[:],
                            v_aug[:, kt, ds((2 * t) * 65, 65)],
                            pt[:, 0:QC],
                            start=(kt == 0),
                            stop=(kt == NKT - 1),
                        )
                        nc.tensor.matmul(
                            oB[:],
                            v_aug[:, kt, ds((2 * t + 1) * 65, 65)],
                            pt[:, QC : 2 * QC],
                            start=(kt == 0),
                            stop=(kt == NKT - 1),
                        )

                    scores_exp(0)
                    for f in fillers.get(0, ()):
                        f()
                    for kt in range(1, NKT):
                        scores_exp(kt)
                        for f in fillers.get(kt, ()):
                            f()
                        attn_v(kt - 1)
                    attn_v(NKT - 1)

                    for o_ps, hh in ((oA, 0), (oB, 1)):
                        # copy PSUM->SBUF immediately (frees the accumulator
                        # bank); the recip/broadcast/mul chain runs off the
                        # critical path on SBUF data
                        o_sb = ounpool.tile([65, QC], F32, tag="oun")
                        nc.vector.tensor_copy(o_sb[:], o_ps[:])
                        rec_full = recpool.tile([64, QC], F32, tag="rec")
                        rec = rec_full[0:1, :]
                        nc.vector.reciprocal(rec[:], o_sb[64:65, :])
                        bc = recpool.tile([64, QC], F32, tag="rec")
                        nc.gpsimd.partition_broadcast(bc[:], rec[:])
                        nc.vector.tensor_mul(
                            oT[ds(hh * 64, 64), t, ds(qc * QC, QC)],
                            o_sb[0:64, :],
                            bc[:],
                        )

            for t in range(NPAIR):
                attention_pair(t)
            # trailing output-projection chunk for the last s-chunk
            for dt2 in range(NDT):
                outproj_dt(NQC - 1, dt2)

    nc.finalize()
    return nc


_NC_CACHE = {}


def _get_nc():
    if "nc" not in _NC_CACHE:
        _NC_CACHE["nc"] = build_attn_core(S=S, D=D, HPC=HPC, HD=HD)
    return _NC_CACHE["nc"]


def _make_in_maps(query, key, value, Wq, bq, Wk, bk, Wv, bv, Wo):
    bf = ml_dtypes.bfloat16
    in_maps = []
    for c in range(N_CORES):
        b, hg = c // 2, c % 2
        sl = slice(hg * DV, (hg + 1) * DV)
        in_maps.append(dict(
            query=np.ascontiguousarray(query[b]).astype(bf),
            key=np.ascontiguousarray(key[b]).astype(bf),
            value=np.ascontiguousarray(value[b]).astype(bf),
            Wq=np.ascontiguousarray(Wq[:, sl]).astype(bf),
            Wk=np.ascontiguousarray(Wk[:, sl]).astype(bf),
            Wv=np.ascontiguousarray(Wv[:, sl]).astype(bf),
            Wo=np.ascontiguousarray(Wo[sl, :]).astype(bf),
            bq=np.ascontiguousarray(bq[sl]).astype(bf),
            bk=np.ascontiguousarray(bk[sl]).astype(bf),
            bv=np.ascontiguousarray(bv[sl]).astype(bf),
        ))
    return in_maps


def _assemble(results, bo):
    out = np.empty((B, S, D), dtype=np.float32)
    for b in range(B):
        part = results[2 * b]["out"] + results[2 * b + 1]["out"]   # [D, S]
        out[b] = part.T + bo
    return out


def run(inputs, trace=False):
    """Run on 8 cores; returns (output, BassKernelResults)."""
    from concourse.bass_utils import run_bass_kernel_spmd

    inputs = {k: np.asarray(v) for k, v in inputs.items()}
    nc = _get_nc()
    in_maps = _make_in_maps(
        inputs["query"], inputs["key"], inputs["value"],
        inputs["Wq"], inputs["bq"], inputs["Wk"], inputs["bk"],
        inputs["Wv"], inputs["bv"], inputs["Wo"],
    )
    res = run_bass_kernel_spmd(
        nc, in_maps, core_ids=list(range(N_CORES)), trace=trace
    )
    out = _assemble(res.results, np.asarray(inputs["bo"], dtype=np.float32))
    return out, res


def kernel(**inputs) -> np.ndarray:
    out, _ = run(inputs, trace=False)
    return out


# revision 9
# speedup vs baseline: 1.2962x; 1.0025x over previous
"""Multi-head attention block (B=4, S=2048, D=1024, H=16) on 8 TRN2 NeuronCores.

Sharding: core c handles batch b = c//2 and head-group hg = c%2 (8 heads,
a 512-wide slice of the qkv projections). No collectives: each core
computes a [D, S] transposed partial of the output projection for its
head group; the host sums the two head-group partials per batch, adds
the output bias, and transposes back to [S, D].

Per-core dataflow (bf16 compute, f32 PSUM accumulation):
  - host pre-casts all big inputs to bf16 (so the device does no casting)
  - xbar transpose-DMA loads X^T [din, s] straight from DRAM
  - Q^T/K^T from projections (dout on partitions); biases folded in as
    ones (x) bias rank-1 matmul updates
  - V in natural [s, dout] layout, augmented with a ones column per head
    (softmax denominators ride along the attn@V matmul as a 65th row)
  - scores^T [k, q] per head via zero-padded K=128 matmuls (uniform
    128x128 tile mode); exp on ACT (PSUM -> SBUF bf16, scale=1/8); O_aug
    accumulated over k tiles in PSUM; normalization via DVE reciprocal +
    GPSIMD partition-broadcast + DVE multiply
  - out^T = Wo^T O^T -> [D, S] f32 -> DMA out
"""

import numpy as np
import ml_dtypes

import concourse.bass as bass
import concourse.bacc as bacc
import concourse.mybir as mybir
from concourse.tile import TileContext
from concourse.bass import ds

F32 = mybir.dt.float32
BF16 = mybir.dt.bfloat16
EXP = mybir.ActivationFunctionType.Exp

B, S, D, H, HD = 4, 2048, 1024, 16, 64
N_CORES = 8
HPC = H // (N_CORES // B)          # heads per core = 8
DV = HPC * HD                      # 512


def build_attn_core(S=2048, D=1024, HPC=8, HD=64):
    DV = HPC * HD            # head-group width
    NPAIR = HPC // 2         # head pairs; DV = NPAIR * 128
    NDT = D // 128           # din tiles
    NKT = S // 128           # key tiles
    QC = 512                 # q chunk
    NQC = S // QC
    SC = 512                 # s chunk for projections
    NSC = S // SC
    SCALE = HD ** -0.5

    nc = bacc.Bacc("TRN2", target_bir_lowering=False)
    q_ext = nc.dram_tensor("query", [S, D], BF16, kind="ExternalInput")
    k_ext = nc.dram_tensor("key", [S, D], BF16, kind="ExternalInput")
    v_ext = nc.dram_tensor("value", [S, D], BF16, kind="ExternalInput")
    wq_ext = nc.dram_tensor("Wq", [D, DV], BF16, kind="ExternalInput")
    wk_ext = nc.dram_tensor("Wk", [D, DV], BF16, kind="ExternalInput")
    wv_ext = nc.dram_tensor("Wv", [D, DV], BF16, kind="ExternalInput")
    wo_ext = nc.dram_tensor("Wo", [DV, D], BF16, kind="ExternalInput")
    bq_ext = nc.dram_tensor("bq", [DV], BF16, kind="ExternalInput")
    bk_ext = nc.dram_tensor("bk", [DV], BF16, kind="ExternalInput")
    bv_ext = nc.dram_tensor("bv", [DV], BF16, kind="ExternalInput")
    out_ext = nc.dram_tensor("out", [D, S], F32, kind="ExternalOutput")

    with TileContext(nc) as tc:
        with (
            tc.tile_pool(name="const", bufs=1) as cpool,
            tc.tile_pool(name="big", bufs=1) as big,
            tc.tile_pool(name="pt", bufs=5) as ptpool,
            tc.tile_pool(name="rec", bufs=2) as recpool,
            tc.tile_pool(name="oun", bufs=4) as ounpool,
            tc.tile_pool(name="stage", bufs=2) as stage,
            tc.tile_pool(name="mmps", bufs=2, space="PSUM") as mmps,
            tc.tile_pool(name="scps", bufs=2, space="PSUM") as scps,
            tc.tile_pool(name="ops", bufs=2, space="PSUM") as opool,
        ):
            # -------- biases / ones first (tiny DMAs; the last matmul of
            # every projection group needs them, so they must not queue
            # behind the big transfers). Zero-padded to 128 partitions so
            # every matmul runs in the same 128x128 tile mode.
            bq_pad = cpool.tile([128, DV], BF16, tag="bqp")
            bk_pad = cpool.tile([128, DV], BF16, tag="bkp")
            bv_pad = cpool.tile([128, DV], BF16, tag="bvp")
            ones_pad = cpool.tile([128, SC], BF16, tag="onesp")
            nc.vector.memset(bq_pad[:], 0.0)
            nc.vector.memset(bk_pad[:], 0.0)
            nc.vector.memset(bv_pad[:], 0.0)
            nc.vector.memset(ones_pad[:], 0.0)
            nc.vector.memset(ones_pad[0:1, :], 1.0)
            nc.sync.dma_start(bq_pad[0:1, :], bq_ext.rearrange("(a n) -> a n", a=1))
            nc.sync.dma_start(bk_pad[0:1, :], bk_ext.rearrange("(a n) -> a n", a=1))
            nc.sync.dma_start(bv_pad[0:1, :], bv_ext.rearrange("(a n) -> a n", a=1))

            # -------- big inputs: emission order = DMA priority ----------
            xqT = big.tile([128, NDT, S], BF16, tag="xqT")
            xkT = big.tile([128, NDT, S], BF16, tag="xkT")
            xvT = big.tile([128, NDT, S], BF16, tag="xqT")  # reuse xqT memory
            wq_sb = big.tile([128, NDT, DV], BF16, tag="wq")
            wk_sb = big.tile([128, NDT, DV], BF16, tag="wk")
            wv_sb = big.tile([128, NDT, DV], BF16, tag="wv")
            wo_sb = big.tile([128, NPAIR, D], BF16, tag="wo")
            nc.sync.dma_start(wq_sb[:], wq_ext.rearrange("(t p) n -> p t n", p=128))
            for dt in range(NDT):
                nc.sync.dma_start_transpose(xqT[:, dt, :], q_ext[:, ds(dt * 128, 128)])
            nc.sync.dma_start(wk_sb[:], wk_ext.rearrange("(t p) n -> p t n", p=128))
            for dt in range(NDT):
                nc.sync.dma_start_transpose(xkT[:, dt, :], k_ext[:, ds(dt * 128, 128)])
            nc.sync.dma_start(wv_sb[:], wv_ext.rearrange("(t p) n -> p t n", p=128))
            for dt in range(NDT):
                nc.sync.dma_start_transpose(xvT[:, dt, :], v_ext[:, ds(dt * 128, 128)])
            nc.sync.dma_start(wo_sb[:], wo_ext.rearrange("(t p) n -> p t n", p=128))

            # -------- persistent SBUF tensors ----------------------------
            # kT is stored twice with complementary halves zeroed, so the
            # scores matmuls can use full K=128 operands (uniform 128x128
            # tile mode, no mode-switch drains): the zero rows of the
            # stationary operand nullify the other head's contribution.
            qT = big.tile([128, NPAIR, S], BF16, tag="qT")
            kTe = big.tile([128, NPAIR, S], BF16, tag="kTe")
            kTo = big.tile([128, NPAIR, S], BF16, tag="kTo")
            v_aug = big.tile([128, NKT, HPC * 65], BF16, tag="vaug")
            oT = big.tile([128, NPAIR, S], BF16, tag="oT")
            nc.gpsimd.memset(kTe[64:128, :, :], 0.0)
            nc.gpsimd.memset(kTo[0:64, :, :], 0.0)

            # -------- work-chunk emitters --------------------------------
            def q_proj_sc(t, sc):
                ps = mmps.tile([128, SC], F32, tag="mm")
                for dk in range(NDT):
                    nc.tensor.matmul(
                        ps[:],
                        wq_sb[:, dk, ds(t * 128, 128)],
                        xqT[:, dk, ds(sc * SC, SC)],
                        start=(dk == 0),
                        stop=False,
                    )
                nc.tensor.matmul(
                    ps[:], bq_pad[:, ds(t * 128, 128)], ones_pad[:],
                    start=False, stop=True,
                )
                nc.vector.tensor_copy(qT[:, t, ds(sc * SC, SC)], ps[:])

            def k_proj_sc(t, sc):
                ps = mmps.tile([128, SC], F32, tag="mm")
                for dk in range(NDT):
                    nc.tensor.matmul(
                        ps[:],
                        wk_sb[:, dk, ds(t * 128, 128)],
                        xkT[:, dk, ds(sc * SC, SC)],
                        start=(dk == 0),
                        stop=False,
                    )
                nc.tensor.matmul(
                    ps[:], bk_pad[:, ds(t * 128, 128)], ones_pad[:],
                    start=False, stop=True,
                )
                nc.vector.tensor_copy(kTe[0:64, t, ds(sc * SC, SC)], ps[0:64, :])
                nc.vector.tensor_copy(kTo[64:128, t, ds(sc * SC, SC)], ps[64:128, :])

            def v_proj_st(st):
                ps = mmps.tile([128, DV], F32, tag="mm")
                for dk in range(NDT):
                    nc.tensor.matmul(
                        ps[:],
                        xvT[:, dk, ds(st * 128, 128)],
                        wv_sb[:, dk, :],
                        start=(dk == 0),
                        stop=False,
                    )
                nc.tensor.matmul(
                    ps[:], ones_pad[:, 0:128], bv_pad[:], start=False, stop=True
                )
                dst = v_aug[:, st, :].rearrange("p (h c) -> p h c", c=65)
                nc.vector.tensor_copy(
                    dst[:, :, 0:64], ps[:].rearrange("p (h c) -> p h c", c=64)
                )
                nc.vector.memset(dst[:, :, 64:65], 1.0)

            def outproj_dt(sc, dt2):
                po = mmps.tile([128, SC], F32, tag="mm")
                for ht in range(NPAIR):
                    nc.tensor.matmul(
                        po[:],
                        wo_sb[:, ht, ds(dt2 * 128, 128)],
                        oT[:, ht, ds(sc * SC, SC)],
                        start=(ht == 0),
                        stop=(ht == NPAIR - 1),
                    )
                ost = stage.tile([128, SC], F32, tag="ostage")
                nc.vector.tensor_copy(ost[:], po[:])
                nc.sync.dma_start(
                    out_ext[ds(dt2 * 128, 128), ds(sc * SC, SC)], ost[:]
                )

            # -------- projections needed before attention(0) -------------
            # all Q pairs up front (xvT reuses xqT memory, so V-proj can
            # only start after the last Q-projection read anyway)
            for t in range(NPAIR):
                for sc in range(NSC):
                    q_proj_sc(t, sc)
            for sc in range(NSC):
                k_proj_sc(0, sc)

            # -------- attention ------------------------------------------
            # ACT (exp) is the long pole; every other PE-work chunk is
            # interleaved into the kt loop as "filler" so the in-order PE
            # queue never parks a long burst in front of the next scores.
            SCALEF = SCALE

            def attention_pair(t):
                for qc in range(NQC):
                    # filler thunks interleaved after each scores/exp step;
                    # lag = how many k-tiles attn@V trails the exp stream
                    # (deep for the very first chunk so attn@V can wait for
                    # the V projection without stalling the exp feed)
                    fillers = {}
                    lag = 2
                    if t == 0 and qc == 0:
                        # V projection interleaved 2 tiles per kt from kt=1;
                        # attn@V trails by 4 k-tiles so v_aug[kt] is always
                        # emitted (and computed) before its consumer
                        lag = 4
                        for st in range(NKT):
                            fillers.setdefault(1 + st // 2, []).append(
                                lambda st=st: v_proj_st(st)
                            )
                    if t + 1 < NPAIR and qc == NQC - 1:
                        for i in range(NSC):
                            fillers[2 + 4 * i] = [
                                lambda tt=t + 1, sc=i: k_proj_sc(tt, sc),
                            ]
                    if t == NPAIR - 1 and qc > 0:
                        for i in range(NDT):
                            fillers[2 * i + 1] = [
                                lambda sc=qc - 1, dt2=i: outproj_dt(sc, dt2),
                            ]

                    oA = opool.tile([65, QC], F32, tag="o")
                    oB = opool.tile([65, QC], F32, tag="o")
                    pts = {}

                    def scores_exp(kt):
                        sct = scps.tile([128, 2 * QC], F32, tag="sc")
                        nc.tensor.matmul(
                            sct[:, 0:QC],
                            kTe[:, t, ds(kt * 128, 128)],
                            qT[:, t, ds(qc * QC, QC)],
                            start=True, stop=True,
                        )
                        nc.tensor.matmul(
                            sct[:, QC : 2 * QC],
                            kTo[:, t, ds(kt * 128, 128)],
                            qT[:, t, ds(qc * QC, QC)],
                            start=True, stop=True,
                        )
                        pt = ptpool.tile([128, 2 * QC], BF16, tag="pt")
                        nc.scalar.activation(pt[:], sct[:], EXP, bias=0.0, scale=SCALEF)
                        pts[kt] = pt

                    def attn_v(kt):
                        pt = pts.pop(kt)
                        nc.tensor.matmul(
                            oA[:],
                            v_aug[:, kt, ds((2 * t) * 65, 65)],
                            pt[:, 0:QC],
                            start=(kt == 0),
                            stop=(kt == NKT - 1),
                        )
                        nc.tensor.matmul(
                            oB[:],
                            v_aug[:, kt, ds((2 * t + 1) * 65, 65)],
                            pt[:, QC : 2 * QC],
                            start=(kt == 0),
                            stop=(kt == NKT - 1),
                        )

                    scores_exp(0)
                    for f in fillers.get(0, ()):
                        f()
                    for kt in range(1, NKT):
                        scores_exp(kt)
                        for f in fillers.get(kt, ()):
                            f()
                        if kt - lag >= 0:
                            attn_v(kt - lag)
                    for kt in range(max(0, NKT - lag), NKT):
                        attn_v(kt)

                    for o_ps, hh in ((oA, 0), (oB, 1)):
                        # copy PSUM->SBUF immediately (frees the accumulator
                        # bank); the recip/broadcast/mul chain runs off the
                        # critical path on SBUF data
                        o_sb = ounpool.tile([65, QC], F32, tag="oun")
                        nc.vector.tensor_copy(o_sb[:], o_ps[:])
                        rec_full = recpool.tile([64, QC], F32, tag="rec")
                        rec = rec_full[0:1, :]
                        nc.vector.reciprocal(rec[:], o_sb[64:65, :])
                        bc = recpool.tile([64, QC], F32, tag="rec")
                        nc.gpsimd.partition_broadcast(bc[:], rec[:])
                        nc.vector.tensor_mul(
                            oT[ds(hh * 64, 64), t, ds(qc * QC, QC)],
                            o_sb[0:64, :],
                            bc[:],
                        )

            for t in range(NPAIR):
                attention_pair(t)
            # trailing output-projection chunk for the last s-chunk
            for dt2 in range(NDT):
                outproj_dt(NQC - 1, dt2)

    nc.finalize()
    return nc


_NC_CACHE = {}


def _get_nc():
    if "nc" not in _NC_CACHE:
        _NC_CACHE["nc"] = build_attn_core(S=S, D=D, HPC=HPC, HD=HD)
    return _NC_CACHE["nc"]


def _make_in_maps(query, key, value, Wq, bq, Wk, bk, Wv, bv, Wo):
    bf = ml_dtypes.bfloat16
    in_maps = []
    for c in range(N_CORES):
        b, hg = c // 2, c % 2
        sl = slice(hg * DV, (hg + 1) * DV)
        in_maps.append(dict(
            query=np.ascontiguousarray(query[b]).astype(bf),
            key=np.ascontiguousarray(key[b]).astype(bf),
            value=np.ascontiguousarray(value[b]).astype(bf),
            Wq=np.ascontiguousarray(Wq[:, sl]).astype(bf),
            Wk=np.ascontiguousarray(Wk[:, sl]).astype(bf),
            Wv=np.ascontiguousarray(Wv[:, sl]).astype(bf),
            Wo=np.ascontiguousarray(Wo[sl, :]).astype(bf),
            bq=np.ascontiguousarray(bq[sl]).astype(bf),
            bk=np.ascontiguousarray(bk[sl]).astype(bf),
            bv=np.ascontiguousarray(bv[sl]).astype(bf),
        ))
    return in_maps


def _assemble(results, bo):
    out = np.empty((B, S, D), dtype=np.float32)
    for b in range(B):
        part = results[2 * b]["out"] + results[2 * b + 1]["out"]   # [D, S]
        out[b] = part.T + bo
    return out


def run(inputs, trace=False):
    """Run on 8 cores; returns (output, BassKernelResults)."""
    from concourse.bass_utils import run_bass_kernel_spmd

    inputs = {k: np.asarray(v) for k, v in inputs.items()}
    nc = _get_nc()
    in_maps = _make_in_maps(
        inputs["query"], inputs["key"], inputs["value"],
        inputs["Wq"], inputs["bq"], inputs["Wk"], inputs["bk"],
        inputs["Wv"], inputs["bv"], inputs["Wo"],
    )
    res = run_bass_kernel_spmd(
        nc, in_maps, core_ids=list(range(N_CORES)), trace=trace
    )
    out = _assemble(res.results, np.asarray(inputs["bo"], dtype=np.float32))
    return out, res


def kernel(**inputs) -> np.ndarray:
    out, _ = run(inputs, trace=False)
    return out


# revision 10
# speedup vs baseline: 1.3259x; 1.0229x over previous
"""Multi-head attention block (B=4, S=2048, D=1024, H=16) on 8 TRN2 NeuronCores.

Sharding: core c handles batch b = c//2 and head-group hg = c%2 (8 heads,
a 512-wide slice of the qkv projections). No collectives: each core
computes a [D, S] transposed partial of the output projection for its
head group; the host sums the two head-group partials per batch, adds
the output bias, and transposes back to [S, D].

Per-core dataflow (bf16 compute, f32 PSUM accumulation):
  - host pre-casts all big inputs to bf16 (so the device does no casting)
  - xbar transpose-DMA loads X^T [din, s] straight from DRAM
  - Q^T/K^T from projections (dout on partitions); biases folded in as
    ones (x) bias rank-1 matmul updates
  - V in natural [s, dout] layout, augmented with a ones column per head
    (softmax denominators ride along the attn@V matmul as a 65th row)
  - scores^T [k, q] per head via zero-padded K=128 matmuls (uniform
    128x128 tile mode); exp on ACT (PSUM -> SBUF bf16, scale=1/8); O_aug
    accumulated over k tiles in PSUM; normalization via DVE reciprocal +
    GPSIMD partition-broadcast + DVE multiply
  - out^T = Wo^T O^T -> [D, S] f32 -> DMA out
"""

import numpy as np
import ml_dtypes

import concourse.bass as bass
import concourse.bacc as bacc
import concourse.mybir as mybir
from concourse.tile import TileContext
from concourse.bass import ds

F32 = mybir.dt.float32
BF16 = mybir.dt.bfloat16
EXP = mybir.ActivationFunctionType.Exp

B, S, D, H, HD = 4, 2048, 1024, 16, 64
N_CORES = 8
HPC = H // (N_CORES // B)          # heads per core = 8
DV = HPC * HD                      # 512


def build_attn_core(S=2048, D=1024, HPC=8, HD=64):
    DV = HPC * HD            # head-group width
    NPAIR = HPC // 2         # head pairs; DV = NPAIR * 128
    NDT = D // 128           # din tiles
    NKT = S // 128           # key tiles
    QC = 512                 # q chunk
    NQC = S // QC
    SC = 512                 # s chunk for projections
    NSC = S // SC
    SCALE = HD ** -0.5

    nc = bacc.Bacc("TRN2", target_bir_lowering=False)
    q_ext = nc.dram_tensor("query", [S, D], BF16, kind="ExternalInput")
    k_ext = nc.dram_tensor("key", [S, D], BF16, kind="ExternalInput")
    v_ext = nc.dram_tensor("value", [S, D], BF16, kind="ExternalInput")
    wq_ext = nc.dram_tensor("Wq", [D, DV], BF16, kind="ExternalInput")
    wk_ext = nc.dram_tensor("Wk", [D, DV], BF16, kind="ExternalInput")
    wv_ext = nc.dram_tensor("Wv", [D, DV], BF16, kind="ExternalInput")
    wo_ext = nc.dram_tensor("Wo", [DV, D], BF16, kind="ExternalInput")
    bq_ext = nc.dram_tensor("bq", [DV], BF16, kind="ExternalInput")
    bk_ext = nc.dram_tensor("bk", [DV], BF16, kind="ExternalInput")
    bv_ext = nc.dram_tensor("bv", [DV], BF16, kind="ExternalInput")
    out_ext = nc.dram_tensor("out", [D, S], F32, kind="ExternalOutput")

    with TileContext(nc) as tc:
        with (
            tc.tile_pool(name="const", bufs=1) as cpool,
            tc.tile_pool(name="big", bufs=1) as big,
            tc.tile_pool(name="pt", bufs=5) as ptpool,
            tc.tile_pool(name="rec", bufs=2) as recpool,
            tc.tile_pool(name="oun", bufs=4) as ounpool,
            tc.tile_pool(name="stage", bufs=2) as stage,
            tc.tile_pool(name="mmps", bufs=2, space="PSUM") as mmps,
            tc.tile_pool(name="scps", bufs=2, space="PSUM") as scps,
            tc.tile_pool(name="ops", bufs=2, space="PSUM") as opool,
        ):
            # -------- biases / ones first (tiny DMAs; the last matmul of
            # every projection group needs them, so they must not queue
            # behind the big transfers). Zero-padded to 128 partitions so
            # every matmul runs in the same 128x128 tile mode.
            bq_pad = cpool.tile([128, DV], BF16, tag="bqp")
            bk_pad = cpool.tile([128, DV], BF16, tag="bkp")
            bv_pad = cpool.tile([128, DV], BF16, tag="bvp")
            ones_pad = cpool.tile([128, SC], BF16, tag="onesp")
            nc.vector.memset(bq_pad[:], 0.0)
            nc.vector.memset(bk_pad[:], 0.0)
            nc.vector.memset(bv_pad[:], 0.0)
            nc.vector.memset(ones_pad[:], 0.0)
            nc.vector.memset(ones_pad[0:1, :], 1.0)
            nc.sync.dma_start(bq_pad[0:1, :], bq_ext.rearrange("(a n) -> a n", a=1))
            nc.sync.dma_start(bk_pad[0:1, :], bk_ext.rearrange("(a n) -> a n", a=1))
            nc.sync.dma_start(bv_pad[0:1, :], bv_ext.rearrange("(a n) -> a n", a=1))

            # -------- big inputs: emission order = DMA priority ----------
            xqT = big.tile([128, NDT, S], BF16, tag="xqT")
            xkT = big.tile([128, NDT, S], BF16, tag="xkT")
            xvT = big.tile([128, NDT, S], BF16, tag="xqT")  # reuse xqT memory
            wq_sb = big.tile([128, NDT, DV], BF16, tag="wq")
            wk_sb = big.tile([128, NDT, DV], BF16, tag="wk")
            wv_sb = big.tile([128, NDT, DV], BF16, tag="wv")
            wo_sb = big.tile([128, NPAIR, D], BF16, tag="wo")
            nc.sync.dma_start(wq_sb[:], wq_ext.rearrange("(t p) n -> p t n", p=128))
            for dt in range(NDT):
                nc.sync.dma_start_transpose(xqT[:, dt, :], q_ext[:, ds(dt * 128, 128)])
            nc.sync.dma_start(wk_sb[:], wk_ext.rearrange("(t p) n -> p t n", p=128))
            for dt in range(NDT):
                nc.sync.dma_start_transpose(xkT[:, dt, :], k_ext[:, ds(dt * 128, 128)])
            nc.sync.dma_start(wv_sb[:], wv_ext.rearrange("(t p) n -> p t n", p=128))
            for dt in range(NDT):
                nc.sync.dma_start_transpose(xvT[:, dt, :], v_ext[:, ds(dt * 128, 128)])
            nc.sync.dma_start(wo_sb[:], wo_ext.rearrange("(t p) n -> p t n", p=128))

            # -------- persistent SBUF tensors ----------------------------
            # kT is stored twice with complementary halves zeroed, so the
            # scores matmuls can use full K=128 operands (uniform 128x128
            # tile mode, no mode-switch drains): the zero rows of the
            # stationary operand nullify the other head's contribution.
            qT = big.tile([128, NPAIR, S], BF16, tag="qT")
            kTe = big.tile([128, NPAIR, S], BF16, tag="kTe")
            kTo = big.tile([128, NPAIR, S], BF16, tag="kTo")
            v_aug = big.tile([128, NKT, HPC * 65], BF16, tag="vaug")
            oT = big.tile([128, NPAIR, S], BF16, tag="oT")
            nc.gpsimd.memset(kTe[64:128, :, :], 0.0)
            nc.gpsimd.memset(kTo[0:64, :, :], 0.0)

            # -------- work-chunk emitters --------------------------------
            def q_proj_sc(t, sc):
                ps = mmps.tile([128, SC], F32, tag="mm")
                for dk in range(NDT):
                    nc.tensor.matmul(
                        ps[:],
                        wq_sb[:, dk, ds(t * 128, 128)],
                        xqT[:, dk, ds(sc * SC, SC)],
                        start=(dk == 0),
                        stop=False,
                    )
                nc.tensor.matmul(
                    ps[:], bq_pad[:, ds(t * 128, 128)], ones_pad[:],
                    start=False, stop=True,
                )
                nc.vector.tensor_copy(qT[:, t, ds(sc * SC, SC)], ps[:])

            def k_proj_sc(t, sc):
                ps = mmps.tile([128, SC], F32, tag="mm")
                for dk in range(NDT):
                    nc.tensor.matmul(
                        ps[:],
                        wk_sb[:, dk, ds(t * 128, 128)],
                        xkT[:, dk, ds(sc * SC, SC)],
                        start=(dk == 0),
                        stop=False,
                    )
                nc.tensor.matmul(
                    ps[:], bk_pad[:, ds(t * 128, 128)], ones_pad[:],
                    start=False, stop=True,
                )
                nc.vector.tensor_copy(kTe[0:64, t, ds(sc * SC, SC)], ps[0:64, :])
                nc.vector.tensor_copy(kTo[64:128, t, ds(sc * SC, SC)], ps[64:128, :])

            def v_proj_st(st):
                ps = mmps.tile([128, DV], F32, tag="mm")
                for dk in range(NDT):
                    nc.tensor.matmul(
                        ps[:],
                        xvT[:, dk, ds(st * 128, 128)],
                        wv_sb[:, dk, :],
                        start=(dk == 0),
                        stop=False,
                    )
                nc.tensor.matmul(
                    ps[:], ones_pad[:, 0:128], bv_pad[:], start=False, stop=True
                )
                dst = v_aug[:, st, :].rearrange("p (h c) -> p h c", c=65)
                nc.vector.tensor_copy(
                    dst[:, :, 0:64], ps[:].rearrange("p (h c) -> p h c", c=64)
                )
                nc.vector.memset(dst[:, :, 64:65], 1.0)

            def outproj_dt(sc, dt2):
                po = mmps.tile([128, SC], F32, tag="mm")
                for ht in range(NPAIR):
                    nc.tensor.matmul(
                        po[:],
                        wo_sb[:, ht, ds(dt2 * 128, 128)],
                        oT[:, ht, ds(sc * SC, SC)],
                        start=(ht == 0),
                        stop=(ht == NPAIR - 1),
                    )
                ost = stage.tile([128, SC], F32, tag="ostage")
                nc.vector.tensor_copy(ost[:], po[:])
                nc.sync.dma_start(
                    out_ext[ds(dt2 * 128, 128), ds(sc * SC, SC)], ost[:]
                )

            # -------- projections needed before attention(0) -------------
            # all Q pairs up front (xvT reuses xqT memory, so V-proj can
            # only start after the last Q-projection read anyway)
            for t in range(NPAIR):
                for sc in range(NSC):
                    q_proj_sc(t, sc)
            for sc in range(NSC):
                k_proj_sc(0, sc)

            # -------- attention ------------------------------------------
            # ACT (exp) is the long pole; every other PE-work chunk is
            # interleaved into the kt loop as "filler" so the in-order PE
            # queue never parks a long burst in front of the next scores.
            SCALEF = SCALE

            def attention_pair(t):
                for qc in range(NQC):
                    # filler thunks interleaved after each scores/exp step;
                    # lag = how many k-tiles attn@V trails the exp stream
                    # (deep for the very first chunk so attn@V can wait for
                    # the V projection without stalling the exp feed)
                    fillers = {}
                    lag = 2
                    if t == 0 and qc == 0:
                        # V projection interleaved 2 tiles per kt from kt=1;
                        # attn@V trails by 4 k-tiles so v_aug[kt] is always
                        # emitted (and computed) before its consumer
                        lag = 4
                        for st in range(NKT):
                            fillers.setdefault(min(1 + st, NKT - 1), []).append(
                                lambda st=st: v_proj_st(st)
                            )
                    if t + 1 < NPAIR and qc == NQC - 1:
                        for i in range(NSC):
                            fillers.setdefault(2 + 4 * i, []).append(
                                lambda tt=t + 1, sc=i: k_proj_sc(tt, sc)
                            )
                    if t == NPAIR - 1 and qc > 0:
                        # even kt only: keeps the DVE queue clear near the
                        # end of the chunk so the O-accumulator release
                        # copies are not delayed
                        for i in range(NDT):
                            fillers.setdefault(2 * i, []).append(
                                lambda sc=qc - 1, dt2=i: outproj_dt(sc, dt2)
                            )

                    oA = opool.tile([65, QC], F32, tag="o")
                    oB = opool.tile([65, QC], F32, tag="o")
                    pts = {}

                    def scores_exp(kt):
                        sct = scps.tile([128, 2 * QC], F32, tag="sc")
                        nc.tensor.matmul(
                            sct[:, 0:QC],
                            kTe[:, t, ds(kt * 128, 128)],
                            qT[:, t, ds(qc * QC, QC)],
                            start=True, stop=True,
                        )
                        nc.tensor.matmul(
                            sct[:, QC : 2 * QC],
                            kTo[:, t, ds(kt * 128, 128)],
                            qT[:, t, ds(qc * QC, QC)],
                            start=True, stop=True,
                        )
                        pt = ptpool.tile([128, 2 * QC], BF16, tag="pt")
                        nc.scalar.activation(pt[:], sct[:], EXP, bias=0.0, scale=SCALEF)
                        pts[kt] = pt

                    def attn_v(kt):
                        pt = pts.pop(kt)
                        nc.tensor.matmul(
                            oA[:],
                            v_aug[:, kt, ds((2 * t) * 65, 65)],
                            pt[:, 0:QC],
                            start=(kt == 0),
                            stop=(kt == NKT - 1),
                        )
                        nc.tensor.matmul(
                            oB[:],
                            v_aug[:, kt, ds((2 * t + 1) * 65, 65)],
                            pt[:, QC : 2 * QC],
                            start=(kt == 0),
                            stop=(kt == NKT - 1),
                        )

                    scores_exp(0)
                    for f in fillers.get(0, ()):
                        f()
                    for kt in range(1, NKT):
                        scores_exp(kt)
                        for f in fillers.get(kt, ()):
                            f()
                        if kt - lag >= 0:
                            attn_v(kt - lag)
                    for kt in range(max(0, NKT - lag), NKT):
                        attn_v(kt)

                    for o_ps, hh in ((oA, 0), (oB, 1)):
                        # copy PSUM->SBUF immediately (frees the accumulator
                        # bank); the recip/broadcast/mul chain runs off the
                        # critical path on SBUF data
                        o_sb = ounpool.tile([65, QC], F32, tag="oun")
                        nc.vector.tensor_copy(o_sb[:], o_ps[:])
                        rec_full = recpool.tile([64, QC], F32, tag="rec")
                        rec = rec_full[0:1, :]
                        nc.vector.reciprocal(rec[:], o_sb[64:65, :])
                        bc = recpool.tile([64, QC], F32, tag="rec")
                        nc.gpsimd.partition_broadcast(bc[:], rec[:])
                        nc.vector.tensor_mul(
                            oT[ds(hh * 64, 64), t, ds(qc * QC, QC)],
                            o_sb[0:64, :],
                            bc[:],
                        )

            for t in range(NPAIR):
                attention_pair(t)
            # trailing output-projection chunk for the last s-chunk
            for dt2 in range(NDT):
                outproj_dt(NQC - 1, dt2)

    nc.finalize()
    return nc


_NC_CACHE = {}


def _get_nc():
    if "nc" not in _NC_CACHE:
        _NC_CACHE["nc"] = build_attn_core(S=S, D=D, HPC=HPC, HD=HD)
    return _NC_CACHE["nc"]


def _make_in_maps(query, key, value, Wq, bq, Wk, bk, Wv, bv, Wo):
    bf = ml_dtypes.bfloat16
    in_maps = []
    for c in range(N_CORES):
        b, hg = c // 2, c % 2
        sl = slice(hg * DV, (hg + 1) * DV)
        in_maps.append(dict(
            query=np.ascontiguousarray(query[b]).astype(bf),
            key=np.ascontiguousarray(key[b]).astype(bf),
            value=np.ascontiguousarray(value[b]).astype(bf),
            Wq=np.ascontiguousarray(Wq[:, sl]).astype(bf),
            Wk=np.ascontiguousarray(Wk[:, sl]).astype(bf),
            Wv=np.ascontiguousarray(Wv[:, sl]).astype(bf),
            Wo=np.ascontiguousarray(Wo[sl, :]).astype(bf),
            bq=np.ascontiguousarray(bq[sl]).astype(bf),
            bk=np.ascontiguousarray(bk[sl]).astype(bf),
            bv=np.ascontiguousarray(bv[sl]).astype(bf),
        ))
    return in_maps


def _assemble(results, bo):
    out = np.empty((B, S, D), dtype=np.float32)
    for b in range(B):
        part = results[2 * b]["out"] + results[2 * b + 1]["out"]   # [D, S]
        out[b] = part.T + bo
    return out


def run(inputs, trace=False):
    """Run on 8 cores; returns (output, BassKernelResults)."""
    from concourse.bass_utils import run_bass_kernel_spmd

    inputs = {k: np.asarray(v) for k, v in inputs.items()}
    nc = _get_nc()
    in_maps = _make_in_maps(
        inputs["query"], inputs["key"], inputs["value"],
        inputs["Wq"], inputs["bq"], inputs["Wk"], inputs["bk"],
        inputs["Wv"], inputs["bv"], inputs["Wo"],
    )
    res = run_bass_kernel_spmd(
        nc, in_maps, core_ids=list(range(N_CORES)), trace=trace
    )
    out = _assemble(res.results, np.asarray(inputs["bo"], dtype=np.float32))
    return out, res


def kernel(**inputs) -> np.ndarray:
    out, _ = run(inputs, trace=False)
    return out
